# revision 7
# baseline (speedup 1.0000x reference)
"""HODLR matvec kernel for 8 TRN2 NeuronCores (Bass/Tile) — v2.

Sharding: node axis split into 8 contiguous slices of 32768 nodes.

v2 redesign (from v1 trace analysis: 186us, ~30us dead zone waiting on a
collective triggered at 58us + LDWEIGHTS-bound projection):

  projection  x is the STATIONARY operand (64-col weight loads, cheap);
              u streams as the moving operand, all levels at once per
              128-node chunk.  Two passes: levels 0-2 first (6.3MB of u)
              so the AllGather triggers at ~26us, then levels 3-7.
              Even/odd chunks run concurrently on col-groups (0,0)/(0,64)
              of the PE array; the halves are merged for free by the
              stacked-identity transpose matmuls.
  tree        bf16 lane-aligned adds on gpsimd (S2..S16 granularities).
  transpose   t^T[b,r] -> t[r,b] stationaries via normal matmuls with a
              stacked identity rhs ([I64;I64]), which also merges the
              even/odd col-group halves.
  expansion   fp8 DoubleRow matmuls, K=256 per instruction; per 512-node
              group one matmul over levels 0-3 (CC-gated) and one over
              levels 4-7 (local) accumulate into one PSUM tile.  ut
              streams interleaved (lvl 0-3 tile i, lvl 4-7 tile i, ...)
              so a late collective only stalls the PE, not the DMA pipe.

DMA priority order on the sync HWDGE queue: x -> u(l0-2) -> u(l3-7) ->
ut interleaved.  corr writes + CC recv ride the scalar queue; the masked
sibling combine runs on gpsimd (strict-FIFO-safe: trigger, tree adds,
combine are its only queue entries).
u/x are fed as fp8e4m3 (u scaled by USCALE; host divides the returned
corrections by USCALE^2).  Host computes diag*x in fp32 and adds.
"""

import sys

sys.path.insert(0, "/opt/trn_rl_repo")

import numpy as np
import ml_dtypes

BF16 = ml_dtypes.bfloat16
FP8 = ml_dtypes.float8_e4m3

B = 64
N = 262144
NCORES = 8
M = N // NCORES          # 32768 nodes per core
R = 64
DEPTH = 8
CH = M // 128            # 256 chunks of 128 nodes
NB7 = M // 1024          # 32 L7 blocks (1024 nodes each)
NG = M // 512            # 64 expansion groups of 512 nodes
USCALE = 64.0

_cached = {}


def _build_bass():
    import concourse.bacc as bacc
    import concourse.tile as tile
    import concourse.mybir as mybir
    from concourse.masks import make_identity
    from contextlib import ExitStack

    BF = mybir.dt.bfloat16
    F8 = mybir.dt.float8e4
    F32 = mybir.dt.float32
    ADD = mybir.AluOpType.add
    MULT = mybir.AluOpType.mult
    DR = mybir.MatmulPerfMode.DoubleRow

    nc = bacc.Bacc(
        "TRN2",
        target_bir_lowering=False,
        debug=False,
        enable_asserts=False,
        num_devices=NCORES,
    )

    xt_d = nc.dram_tensor("xt", [128, CH, B], F8, kind="ExternalInput").ap()
    ua_d = nc.dram_tensor("ua", [128, CH, 192], F8, kind="ExternalInput").ap()
    ub_d = nc.dram_tensor("ub", [128, CH, 320], F8, kind="ExternalInput").ap()
    ut_d = nc.dram_tensor("ut", [4, 128, M], F8, kind="ExternalInput").ap()
    mA_d = nc.dram_tensor("maskA", [128, 8, B], BF, kind="ExternalInput").ap()
    mB_d = nc.dram_tensor("maskB", [64, 8, B], BF, kind="ExternalInput").ap()
    corr_d = nc.dram_tensor("corr", [B, M], F8, kind="ExternalOutput").ap()

    with tile.TileContext(nc) as tc, ExitStack() as ctx:
        const = ctx.enter_context(tc.tile_pool(name="const", bufs=1))
        treep = ctx.enter_context(tc.tile_pool(name="treep", bufs=1))
        statp = ctx.enter_context(tc.tile_pool(name="statp", bufs=1))
        yp = ctx.enter_context(tc.tile_pool(name="yp", bufs=2))
        pp2 = ctx.enter_context(tc.tile_pool(name="pp2", bufs=2, space="PSUM"))
        tpp = ctx.enter_context(tc.tile_pool(name="tpp", bufs=2, space="PSUM"))
        dram = ctx.enter_context(tc.tile_pool(name="dram", bufs=1, space="DRAM"))

        # ---------------- constants + x ----------------
        xt = const.tile([128, CH, B], F8, tag="xt")
        for xq in range(4):
            nc.sync.dma_start(
                xt[:, 64 * xq : 64 * (xq + 1), :],
                xt_d[:, 64 * xq : 64 * (xq + 1), :],
            )
        # stacked identity [I64; I64]: transpose-merge rhs
        idS = const.tile([128, 64], BF, tag="idS")
        make_identity(nc, idS[0:64, :])
        make_identity(nc, idS[64:128, :])
        mA = const.tile([128, 8, B], BF, tag="mA")
        nc.scalar.dma_start(mA[:], mA_d[:])
        mB = const.tile([64, 8, B], BF, tag="mB")
        nc.scalar.dma_start(mB[:], mB_d[:])
        b_in = dram.tile([192, B], BF, tag="b_in")
        b_out = dram.tile([8, 192, B], BF, tag="b_out", addr_space="Shared")

        # round-robin for PSUM->SBUF drains (gpsimd cannot read PSUM)
        _cc = [0]

        def drain(out, in_):
            e = "vs"[_cc[0] % 2]
            _cc[0] += 1
            if e == "v":
                nc.vector.tensor_copy(out, in_)
            else:
                nc.scalar.copy(out, in_)

        # ---------------- pass 1: levels 0-2 projection ----------------
        # t012^T[b, l*64+r] accumulated over ALL chunks (levels 0-2 need
        # the full-slice contraction; sibling data comes via AllGather).
        with tc.tile_pool(name="pp1", bufs=1, space="PSUM") as pp1, \
             tc.tile_pool(name="uap", bufs=2) as uap:
            p1 = pp1.tile([128, 192], F32, tag="p1")
            UAT = 4
            for t_ in range(UAT):
                ua_t = uap.tile([128, 64, 192], F8, tag="ua", name=f"ua{t_}")
                if t_ == 0:
                    for sq in range(4):
                        nc.sync.dma_start(
                            ua_t[:, 16 * sq : 16 * (sq + 1), :],
                            ua_d[:, 16 * sq : 16 * (sq + 1), :],
                        )
                else:
                    nc.sync.dma_start(
                        ua_t[:], ua_d[:, 64 * t_ : 64 * (t_ + 1), :]
                    )
                for m in range(64):
                    ck = 64 * t_ + m
                    half = ck % 2
                    nc.tensor.matmul(
                        p1[64 * half : 64 * half + 64, :],
                        xt[:, ck, :],
                        ua_t[:, m, :],
                        start=(ck < 2),
                        stop=(ck >= CH - 2),
                    )
            s012 = treep.tile([128, 192], BF, tag="s012")
            with tc.high_priority():
                nc.vector.tensor_copy(s012[:], p1[:])

        # transpose-merge t012 -> b_in layout [l*64+r, b]; the whole
        # chain down to the CC trigger is high-priority so the Tile
        # scheduler does not defer any link behind pass-2 work.
        with tc.high_priority():
            pAB_t = tpp.tile([128, 2, 64], F32, tag="tp", name="tp_AB")
            pAB = pAB_t[:, 0, :]
            nc.tensor.matmul(pAB[0:64, :], s012[:, 0:64], idS[:], start=True, stop=True)
            nc.tensor.matmul(pAB[64:128, :], s012[:, 64:128], idS[:], start=True, stop=True)
            pC_t = tpp.tile([128, 2, 64], F32, tag="tp", name="tp_C")
            pC = pC_t[0:64, 0, :]
            nc.tensor.matmul(pC[:], s012[:, 128:192], idS[:], start=True, stop=True)
            bsA = treep.tile([128, B], BF, tag="bsA")
            nc.vector.tensor_copy(bsA[:], pAB[:])
            bsB = treep.tile([64, B], BF, tag="bsB")
            nc.vector.tensor_copy(bsB[:], pC[:])
            nc.scalar.dma_start(b_in[0:128, :], bsA[:])
            nc.scalar.dma_start(b_in[128:192, :], bsB[:])
            nc.gpsimd.collective_compute(
                "AllGather",
                mybir.AluOpType.bypass,
                replica_groups=[list(range(NCORES))],
                ins=[b_in.opt()],
                outs=[b_out.opt()],
            )
        # receive path: scalar DMAs now; masked combine on gpsimd is
        # emitted AFTER the tree adds so it doesn't block them in FIFO.
        recvA = statp.tile([128, 8, B], BF, tag="recvA")
        recvB = statp.tile([64, 8, B], BF, tag="recvB")
        for k in range(8):
            nc.scalar.dma_start(recvA[:, k, :], b_out[k, 0:128, :])
            nc.scalar.dma_start(recvB[:, k, :], b_out[k, 128:192, :])

        # ---------------- pass 2: levels 3-7 projection + tree ----------
        # pass-2 column layout: l3@0:64 l4@64:128 l5@128:192 l6@192:256
        # l7@256:320.  s1[j] rows 0:64 = even-chunk partial of block j,
        # rows 64:128 = odd-chunk partial (merged later by transposes).
        S1, S2, S4, S8, S16 = [], [], [], [], []
        statA = [None] * NB7
        statB = [None] * 2

        def tree_add(lst, src, j, width, tag):
            g2 = treep.tile([128, width], BF, tag=f"{tag}_{j}")
            nc.gpsimd.tensor_tensor(
                g2[:], src[2 * j][:, 0:width], src[2 * j + 1][:, 0:width], op=ADD
            )
            lst.append(g2)

        def emit_statA(m7):
            # build [128, 2, B] fp8 stationary for levels 4-7 of block m7
            ps = tpp.tile([128, 2, 64], F32, tag="tp", name=f"tA4_{m7}")
            nc.tensor.matmul(
                ps[0:64, 0, :], S8[(m7 // 8) ^ 1][:, 64:128], idS[:],
                start=True, stop=True,
            )
            nc.tensor.matmul(
                ps[64:128, 0, :], S4[(m7 // 4) ^ 1][:, 128:192], idS[:],
                start=True, stop=True,
            )
            nc.tensor.matmul(
                ps[0:64, 1, :], S2[(m7 // 2) ^ 1][:, 192:256], idS[:],
                start=True, stop=True,
            )
            nc.tensor.matmul(
                ps[64:128, 1, :], S1[m7 ^ 1][:, 256:320], idS[:],
                start=True, stop=True,
            )
            s = statp.tile([128, 2, B], F8, tag=f"sA_{m7}")
            drain(s[:], ps[:])
            statA[m7] = s

        def emit_statB_l3(m3):
            # transpose l3 sibling into a half-stationary; tallA/tallB
            # columns are filled in after the collective combine.
            ps_t = tpp.tile([128, 2, 64], F32, tag="tp", name=f"tB3_{m3}")
            ps = ps_t[0:64, 0, :]
            nc.tensor.matmul(
                ps[:], S16[m3 ^ 1][:, 0:64], idS[:], start=True, stop=True
            )
            s = statp.tile([128, 2, B], F8, tag=f"sB_{m3}")
            nc.vector.tensor_copy(s[64:128, 1, :], ps[:])
            statB[m3] = s

        UBT = 8
        ubp = ctx.enter_context(tc.tile_pool(name="ubp", bufs=2))
        for t_ in range(UBT):
            ub_t = ubp.tile([128, 32, 320], F8, tag="ub", name=f"ub{t_}")
            nc.sync.dma_start(ub_t[:], ub_d[:, 32 * t_ : 32 * (t_ + 1), :])
            for blk in range(4):
                j = 4 * t_ + blk
                p2 = pp2.tile([128, 320], F32, tag="p2", name=f"p2_{j}")
                for ck8 in range(8):
                    ck = 8 * j + ck8
                    half = ck % 2
                    nc.tensor.matmul(
                        p2[64 * half : 64 * half + 64, :],
                        xt[:, ck, :],
                        ub_t[:, 8 * blk + ck8, :],
                        start=(ck8 < 2),
                        stop=(ck8 >= 6),
                    )
                s1 = treep.tile([128, 320], BF, tag=f"S1_{j}")
                drain(s1[:], p2[:])
                S1.append(s1)
                if j % 2 == 1:
                    tree_add(S2, S1, j // 2, 256, "S2")
                if j % 4 == 3:
                    tree_add(S4, S2, j // 4, 192, "S4")
                if j % 8 == 7:
                    tree_add(S8, S4, j // 8, 128, "S8")
                if j % 16 == 15:
                    tree_add(S16, S8, j // 16, 64, "S16")
                if j == 15:
                    # S8[0..1], S4[0..3], S2[0..7], S1[0..15] ready
                    for m7 in range(16):
                        emit_statA(m7)
                    emit_statB_l3(1)
                if j == 31:
                    for m7 in range(16, 32):
                        emit_statA(m7)
                    emit_statB_l3(0)

        # masked sibling combine for levels 0-2 (gpsimd; after tree adds)
        mskA = statp.tile([128, 8, B], BF, tag="mskA")
        mskB = statp.tile([64, 8, B], BF, tag="mskB")
        nc.gpsimd.tensor_tensor(mskA[:], recvA[:], mA[:], op=MULT)
        nc.gpsimd.tensor_tensor(mskB[:], recvB[:], mB[:], op=MULT)
        nc.gpsimd.tensor_tensor(
            mskA[:, 0:4, :], mskA[:, 0:4, :], mskA[:, 4:8, :], op=ADD
        )
        nc.gpsimd.tensor_tensor(
            mskB[:, 0:4, :], mskB[:, 0:4, :], mskB[:, 4:8, :], op=ADD
        )
        nc.gpsimd.tensor_tensor(
            mskA[:, 0:2, :], mskA[:, 0:2, :], mskA[:, 2:4, :], op=ADD
        )
        nc.gpsimd.tensor_tensor(
            mskB[:, 0:2, :], mskB[:, 0:2, :], mskB[:, 2:4, :], op=ADD
        )
        tallA = statp.tile([128, B], BF, tag="tallA")
        tallB = statp.tile([64, B], BF, tag="tallB")
        nc.gpsimd.tensor_tensor(tallA[:], mskA[:, 0, :], mskA[:, 1, :], op=ADD)
        nc.gpsimd.tensor_tensor(tallB[:], mskB[:, 0, :], mskB[:, 1, :], op=ADD)
        for m3 in range(2):
            nc.scalar.copy(statB[m3][:, 0, :], tallA[:])
            nc.scalar.copy(statB[m3][0:64, 1, :], tallB[:])

        # ---------------- expansion (fp8 DoubleRow), CC-tolerant split ----
        # pass L: levels 4-7 (fully local) stream first; per group one DR
        # matmul -> PSUM -> fp8 partial in SBUF.  pass C: levels 0-3 ut
        # tiles stream second and stay resident, so when the collective
        # lands (barrier-gated, ~85-105us) only 64 short matmuls + adds
        # remain.  Final add y = yL + psC runs on DVE (direct) or
        # ACT-copy + gpsimd-add, alternating, to spread engine load.
        utap = ctx.enter_context(tc.tile_pool(name="utap", bufs=2))
        utbp = ctx.enter_context(tc.tile_pool(name="utbp", bufs=8))
        ylp = ctx.enter_context(tc.tile_pool(name="ylp", bufs=1))
        scrp = ctx.enter_context(tc.tile_pool(name="scrp", bufs=4))
        yL = [None] * NG
        with tc.tile_pool(name="epL", bufs=4, space="PSUM") as epL:
            for i in range(8):
                uta = utap.tile([128, 2, 4096], F8, tag="uta", name=f"uta{i}")
                for ff in range(2):
                    nc.sync.dma_start(
                        uta[:, ff, :], ut_d[2 + ff, :, 4096 * i : 4096 * (i + 1)]
                    )
                for gg in range(8):
                    g = 8 * i + gg
                    sl = slice(512 * gg, 512 * (gg + 1))
                    psL = epL.tile([B, 512], F32, tag="expL", name=f"psL{g}")
                    nc.tensor.matmul(
                        psL[:], statA[g // 2][:], uta[:, :, sl],
                        start=True, stop=True, perf_mode=DR,
                    )
                    yl = ylp.tile([B, 512], F8, tag=f"yL_{g}")
                    drain(yl[:], psL[:])
                    yL[g] = yl
        # pass C: levels 0-3; ut tiles stay resident in SBUF
        utb = []
        for i in range(8):
            t_ = utbp.tile([128, 2, 4096], F8, tag="utb", name=f"utb{i}")
            for ff in range(2):
                nc.sync.dma_start(
                    t_[:, ff, :], ut_d[ff, :, 4096 * i : 4096 * (i + 1)]
                )
            utb.append(t_)
        with tc.tile_pool(name="epC", bufs=4, space="PSUM") as epC:
            for i in range(8):
                y_t = yp.tile([B, 4096], F8, tag="y", name=f"y{i}")
                for gg in range(8):
                    g = 8 * i + gg
                    sl = slice(512 * gg, 512 * (gg + 1))
                    psC = epC.tile([B, 512], F32, tag="expC", name=f"psC{g}")
                    nc.tensor.matmul(
                        psC[:], statB[g // 32][:], utb[i][:, :, sl],
                        start=True, stop=True, perf_mode=DR,
                    )
                    if g % 2 == 0:
                        nc.vector.tensor_tensor(
                            y_t[:, sl], yL[g][:], psC[:], op=ADD
                        )
                    else:
                        sc = scrp.tile([B, 512], BF, tag="scr", name=f"scr{g}")
                        nc.scalar.copy(sc[:], psC[:])
                        nc.gpsimd.tensor_tensor(
                            y_t[:, sl], yL[g][:], sc[:], op=ADD
                        )
                nc.scalar.dma_start(
                    corr_d[:, 4096 * i : 4096 * (i + 1)], y_t[:]
                )

    nc.compile()
    return nc


def _pack_inputs(x, diag, u):
    """Build per-core input maps. x (B,N,1) f32, u (DEPTH,N,R) f32."""
    in_maps = []
    x2 = np.asarray(x).reshape(B, N)
    u3 = np.asarray(u)
    for c in range(NCORES):
        base = c * M
        xsl = x2[:, base : base + M]                      # (B, M)
        usl = u3[:, base : base + M, :] * USCALE          # (8, M, 64)
        xt = np.ascontiguousarray(
            xsl.T.reshape(CH, 128, B).transpose(1, 0, 2)
        ).astype(FP8)                                     # [128, CH, B]
        ua = np.ascontiguousarray(
            usl[0:3].transpose(1, 0, 2).reshape(M, 192)
            .reshape(CH, 128, 192).transpose(1, 0, 2)
        ).astype(FP8)                                     # [128, CH, 192]
        ub = np.ascontiguousarray(
            usl[3:8].transpose(1, 0, 2).reshape(M, 320)
            .reshape(CH, 128, 320).transpose(1, 0, 2)
        ).astype(FP8)                                     # [128, CH, 320]
        utp = np.ascontiguousarray(
            usl.transpose(0, 2, 1).reshape(4, 128, M)
        ).astype(FP8)                                     # [4, 128, M]
        # masks: mask[d, l] = 1 iff this core c is in the level-l sibling
        # block of destination core d.
        mA = np.zeros((128, 8, B), dtype=BF16)
        mB = np.zeros((64, 8, B), dtype=BF16)
        for d in range(8):
            if (c // 4) == ((d // 4) ^ 1):
                mA[0:64, d, :] = 1.0   # level 0
            if (c // 2) == ((d // 2) ^ 1):
                mA[64:128, d, :] = 1.0  # level 1
            if c == d ^ 1:
                mB[:, d, :] = 1.0       # level 2
        in_maps.append(
            {"xt": xt, "ua": ua, "ub": ub, "ut": utp, "maskA": mA, "maskB": mB}
        )
    return in_maps


last_results = None


def kernel(x, diag, u):
    global last_results
    from concourse.bass_utils import run_bass_kernel_spmd

    if "nc" not in _cached:
        _cached["nc"] = _build_bass()
    nc = _cached["nc"]

    in_maps = _pack_inputs(x, diag, u)
    res = run_bass_kernel_spmd(nc, in_maps, core_ids=list(range(NCORES)))
    last_results = res

    x2 = np.asarray(x, dtype=np.float32).reshape(B, N)
    d2 = np.asarray(diag, dtype=np.float32).reshape(1, N)
    y = d2 * x2
    inv = 1.0 / (USCALE * USCALE)
    for c in range(NCORES):
        corr = np.asarray(res.results[c]["corr"]).astype(np.float32)
        y[:, c * M : (c + 1) * M] += corr * inv
    return y.reshape(B, N, 1).astype(np.float32)


# revision 8
# speedup vs baseline: 1.1954x; 1.1954x over previous
"""HODLR matvec kernel for 8 TRN2 NeuronCores (Bass/Tile) — v2.

Sharding: node axis split into 8 contiguous slices of 32768 nodes.

v2 redesign (from v1 trace analysis: 186us, ~30us dead zone waiting on a
collective triggered at 58us + LDWEIGHTS-bound projection):

  projection  x is the STATIONARY operand (64-col weight loads, cheap);
              u streams as the moving operand, all levels at once per
              128-node chunk.  Two passes: levels 0-2 first (6.3MB of u)
              so the AllGather triggers at ~26us, then levels 3-7.
              Even/odd chunks run concurrently on col-groups (0,0)/(0,64)
              of the PE array; the halves are merged for free by the
              stacked-identity transpose matmuls.
  tree        bf16 lane-aligned adds on gpsimd (S2..S16 granularities).
  transpose   t^T[b,r] -> t[r,b] stationaries via normal matmuls with a
              stacked identity rhs ([I64;I64]), which also merges the
              even/odd col-group halves.
  expansion   fp8 DoubleRow matmuls, K=256 per instruction; per 512-node
              group one matmul over levels 0-3 (CC-gated) and one over
              levels 4-7 (local) accumulate into one PSUM tile.  ut
              streams interleaved (lvl 0-3 tile i, lvl 4-7 tile i, ...)
              so a late collective only stalls the PE, not the DMA pipe.

DMA priority order on the sync HWDGE queue: x -> u(l0-2) -> u(l3-7) ->
ut interleaved.  corr writes + CC recv ride the scalar queue; the masked
sibling combine runs on gpsimd (strict-FIFO-safe: trigger, tree adds,
combine are its only queue entries).
u/x are fed as fp8e4m3 (u scaled by USCALE; host divides the returned
corrections by USCALE^2).  Host computes diag*x in fp32 and adds.
"""

import sys

sys.path.insert(0, "/opt/trn_rl_repo")

import numpy as np
import ml_dtypes

BF16 = ml_dtypes.bfloat16
FP8 = ml_dtypes.float8_e4m3

B = 64
N = 262144
NCORES = 8
M = N // NCORES          # 32768 nodes per core
R = 64
DEPTH = 8
CH = M // 128            # 256 chunks of 128 nodes
NB7 = M // 1024          # 32 L7 blocks (1024 nodes each)
NG = M // 512            # 64 expansion groups of 512 nodes
USCALE = 64.0

_cached = {}


def _build_bass():
    import concourse.bacc as bacc
    import concourse.tile as tile
    import concourse.mybir as mybir
    from concourse.masks import make_identity
    from contextlib import ExitStack

    BF = mybir.dt.bfloat16
    F8 = mybir.dt.float8e4
    F32 = mybir.dt.float32
    ADD = mybir.AluOpType.add
    MULT = mybir.AluOpType.mult
    DR = mybir.MatmulPerfMode.DoubleRow

    nc = bacc.Bacc(
        "TRN2",
        target_bir_lowering=False,
        debug=False,
        enable_asserts=False,
        num_devices=NCORES,
    )

    xt_d = nc.dram_tensor("xt", [128, CH, B], F8, kind="ExternalInput").ap()
    ua_d = nc.dram_tensor("ua", [128, CH, 192], F8, kind="ExternalInput").ap()
    ub_d = nc.dram_tensor("ub", [128, CH, 320], F8, kind="ExternalInput").ap()
    ut_d = nc.dram_tensor("ut", [4, 128, M], F8, kind="ExternalInput").ap()
    mA_d = nc.dram_tensor("maskA", [128, 8, B], BF, kind="ExternalInput").ap()
    mB_d = nc.dram_tensor("maskB", [64, 8, B], BF, kind="ExternalInput").ap()
    corr_d = nc.dram_tensor("corr", [B, M], F8, kind="ExternalOutput").ap()

    with tile.TileContext(nc) as tc, ExitStack() as ctx:
        const = ctx.enter_context(tc.tile_pool(name="const", bufs=1))
        treep = ctx.enter_context(tc.tile_pool(name="treep", bufs=1))
        statp = ctx.enter_context(tc.tile_pool(name="statp", bufs=1))
        yp = ctx.enter_context(tc.tile_pool(name="yp", bufs=2))
        pp2 = ctx.enter_context(tc.tile_pool(name="pp2", bufs=2, space="PSUM"))
        tpp = ctx.enter_context(tc.tile_pool(name="tpp", bufs=2, space="PSUM"))
        dram = ctx.enter_context(tc.tile_pool(name="dram", bufs=1, space="DRAM"))

        # ---------------- constants + x ----------------
        xt = const.tile([128, CH, B], F8, tag="xt")
        for xq in range(4):
            nc.sync.dma_start(
                xt[:, 64 * xq : 64 * (xq + 1), :],
                xt_d[:, 64 * xq : 64 * (xq + 1), :],
            )
        # stacked identity [I64; I64]: transpose-merge rhs
        idS = const.tile([128, 64], BF, tag="idS")
        make_identity(nc, idS[0:64, :])
        make_identity(nc, idS[64:128, :])
        mA = const.tile([128, 8, B], BF, tag="mA")
        nc.scalar.dma_start(mA[:], mA_d[:])
        mB = const.tile([64, 8, B], BF, tag="mB")
        nc.scalar.dma_start(mB[:], mB_d[:])
        b_in = dram.tile([192, B], BF, tag="b_in")
        b_out = dram.tile([8, 192, B], BF, tag="b_out", addr_space="Shared")

        # round-robin for PSUM->SBUF drains (gpsimd cannot read PSUM)
        _cc = [0]

        def drain(out, in_):
            e = "vs"[_cc[0] % 2]
            _cc[0] += 1
            if e == "v":
                nc.vector.tensor_copy(out, in_)
            else:
                nc.scalar.copy(out, in_)

        # ---------------- pass 1: levels 0-2 projection ----------------
        # t012^T[b, l*64+r] accumulated over ALL chunks (levels 0-2 need
        # the full-slice contraction; sibling data comes via AllGather).
        with tc.tile_pool(name="pp1", bufs=1, space="PSUM") as pp1, \
             tc.tile_pool(name="uap", bufs=2) as uap:
            p1 = pp1.tile([128, 192], F32, tag="p1")
            UAT = 4
            for t_ in range(UAT):
                ua_t = uap.tile([128, 64, 192], F8, tag="ua", name=f"ua{t_}")
                if t_ == 0:
                    for sq in range(4):
                        nc.sync.dma_start(
                            ua_t[:, 16 * sq : 16 * (sq + 1), :],
                            ua_d[:, 16 * sq : 16 * (sq + 1), :],
                        )
                else:
                    nc.sync.dma_start(
                        ua_t[:], ua_d[:, 64 * t_ : 64 * (t_ + 1), :]
                    )
                for m in range(64):
                    ck = 64 * t_ + m
                    half = ck % 2
                    nc.tensor.matmul(
                        p1[64 * half : 64 * half + 64, :],
                        xt[:, ck, :],
                        ua_t[:, m, :],
                        start=(ck < 2),
                        stop=(ck >= CH - 2),
                    )
            s012 = treep.tile([128, 192], BF, tag="s012")
            with tc.high_priority():
                nc.vector.tensor_copy(s012[:], p1[:])

        # transpose-merge t012 -> b_in layout [l*64+r, b]; the whole
        # chain down to the CC trigger is high-priority so the Tile
        # scheduler does not defer any link behind pass-2 work.
        with tc.high_priority():
            pAB_t = tpp.tile([128, 2, 64], F32, tag="tp", name="tp_AB")
            pAB = pAB_t[:, 0, :]
            nc.tensor.matmul(pAB[0:64, :], s012[:, 0:64], idS[:], start=True, stop=True)
            nc.tensor.matmul(pAB[64:128, :], s012[:, 64:128], idS[:], start=True, stop=True)
            pC_t = tpp.tile([128, 2, 64], F32, tag="tp", name="tp_C")
            pC = pC_t[0:64, 0, :]
            nc.tensor.matmul(pC[:], s012[:, 128:192], idS[:], start=True, stop=True)
            bsA = treep.tile([128, B], BF, tag="bsA")
            nc.vector.tensor_copy(bsA[:], pAB[:])
            bsB = treep.tile([64, B], BF, tag="bsB")
            nc.vector.tensor_copy(bsB[:], pC[:])
            nc.scalar.dma_start(b_in[0:128, :], bsA[:])
            nc.scalar.dma_start(b_in[128:192, :], bsB[:])
            nc.gpsimd.collective_compute(
                "AllGather",
                mybir.AluOpType.bypass,
                replica_groups=[list(range(NCORES))],
                ins=[b_in.opt()],
                outs=[b_out.opt()],
            )
        # receive path: scalar DMAs now; masked combine on gpsimd is
        # emitted AFTER the tree adds so it doesn't block them in FIFO.
        recvA = statp.tile([128, 8, B], BF, tag="recvA")
        recvB = statp.tile([64, 8, B], BF, tag="recvB")
        for k in range(8):
            nc.scalar.dma_start(recvA[:, k, :], b_out[k, 0:128, :])
            nc.scalar.dma_start(recvB[:, k, :], b_out[k, 128:192, :])

        # ---------------- pass 2: levels 3-7 projection + tree ----------
        # pass-2 column layout: l3@0:64 l4@64:128 l5@128:192 l6@192:256
        # l7@256:320.  s1[j] rows 0:64 = even-chunk partial of block j,
        # rows 64:128 = odd-chunk partial (merged later by transposes).
        S1, S2, S4, S8, S16 = [], [], [], [], []
        statA = [None] * NB7
        statB = [None] * 2

        def tree_add(lst, src, j, width, tag):
            g2 = treep.tile([128, width], BF, tag=f"{tag}_{j}")
            nc.gpsimd.tensor_tensor(
                g2[:], src[2 * j][:, 0:width], src[2 * j + 1][:, 0:width], op=ADD
            )
            lst.append(g2)

        def emit_statA(m7):
            # build [128, 2, B] fp8 stationary for levels 4-7 of block m7
            ps = tpp.tile([128, 2, 64], F32, tag="tp", name=f"tA4_{m7}")
            nc.tensor.matmul(
                ps[0:64, 0, :], S8[(m7 // 8) ^ 1][:, 64:128], idS[:],
                start=True, stop=True,
            )
            nc.tensor.matmul(
                ps[64:128, 0, :], S4[(m7 // 4) ^ 1][:, 128:192], idS[:],
                start=True, stop=True,
            )
            nc.tensor.matmul(
                ps[0:64, 1, :], S2[(m7 // 2) ^ 1][:, 192:256], idS[:],
                start=True, stop=True,
            )
            nc.tensor.matmul(
                ps[64:128, 1, :], S1[m7 ^ 1][:, 256:320], idS[:],
                start=True, stop=True,
            )
            s = statp.tile([128, 2, B], F8, tag=f"sA_{m7}")
            drain(s[:], ps[:])
            statA[m7] = s

        def emit_statB_l3(m3):
            # transpose l3 sibling into a half-stationary; tallA/tallB
            # columns are filled in after the collective combine.
            ps_t = tpp.tile([128, 2, 64], F32, tag="tp", name=f"tB3_{m3}")
            ps = ps_t[0:64, 0, :]
            nc.tensor.matmul(
                ps[:], S16[m3 ^ 1][:, 0:64], idS[:], start=True, stop=True
            )
            s = statp.tile([128, 2, B], F8, tag=f"sB_{m3}")
            nc.vector.tensor_copy(s[64:128, 1, :], ps[:])
            statB[m3] = s

        UBT = 8
        ubp = ctx.enter_context(tc.tile_pool(name="ubp", bufs=2))
        for t_ in range(UBT):
            ub_t = ubp.tile([128, 32, 320], F8, tag="ub", name=f"ub{t_}")
            nc.sync.dma_start(ub_t[:], ub_d[:, 32 * t_ : 32 * (t_ + 1), :])
            for blk in range(4):
                j = 4 * t_ + blk
                p2 = pp2.tile([128, 320], F32, tag="p2", name=f"p2_{j}")
                for ck8 in range(8):
                    ck = 8 * j + ck8
                    half = ck % 2
                    nc.tensor.matmul(
                        p2[64 * half : 64 * half + 64, :],
                        xt[:, ck, :],
                        ub_t[:, 8 * blk + ck8, :],
                        start=(ck8 < 2),
                        stop=(ck8 >= 6),
                    )
                s1 = treep.tile([128, 320], BF, tag=f"S1_{j}")
                drain(s1[:], p2[:])
                S1.append(s1)
                if j % 2 == 1:
                    tree_add(S2, S1, j // 2, 256, "S2")
                if j % 4 == 3:
                    tree_add(S4, S2, j // 4, 192, "S4")
                if j % 8 == 7:
                    tree_add(S8, S4, j // 8, 128, "S8")
                if j % 16 == 15:
                    tree_add(S16, S8, j // 16, 64, "S16")
                if j == 15:
                    # S8[0..1], S4[0..3], S2[0..7], S1[0..15] ready
                    for m7 in range(16):
                        emit_statA(m7)
                    emit_statB_l3(1)
                if j == 31:
                    for m7 in range(16, 32):
                        emit_statA(m7)
                    emit_statB_l3(0)

        # masked sibling combine for levels 0-2 (gpsimd; after tree adds)
        wait_cc = tc.tile_wait_until(0.085)
        wait_cc.__enter__()
        mskA = statp.tile([128, 8, B], BF, tag="mskA")
        mskB = statp.tile([64, 8, B], BF, tag="mskB")
        nc.gpsimd.tensor_tensor(mskA[:], recvA[:], mA[:], op=MULT)
        nc.gpsimd.tensor_tensor(mskB[:], recvB[:], mB[:], op=MULT)
        nc.gpsimd.tensor_tensor(
            mskA[:, 0:4, :], mskA[:, 0:4, :], mskA[:, 4:8, :], op=ADD
        )
        nc.gpsimd.tensor_tensor(
            mskB[:, 0:4, :], mskB[:, 0:4, :], mskB[:, 4:8, :], op=ADD
        )
        nc.gpsimd.tensor_tensor(
            mskA[:, 0:2, :], mskA[:, 0:2, :], mskA[:, 2:4, :], op=ADD
        )
        nc.gpsimd.tensor_tensor(
            mskB[:, 0:2, :], mskB[:, 0:2, :], mskB[:, 2:4, :], op=ADD
        )
        tallA = statp.tile([128, B], BF, tag="tallA")
        tallB = statp.tile([64, B], BF, tag="tallB")
        nc.gpsimd.tensor_tensor(tallA[:], mskA[:, 0, :], mskA[:, 1, :], op=ADD)
        nc.gpsimd.tensor_tensor(tallB[:], mskB[:, 0, :], mskB[:, 1, :], op=ADD)
        for m3 in range(2):
            nc.scalar.copy(statB[m3][:, 0, :], tallA[:])
            nc.scalar.copy(statB[m3][0:64, 1, :], tallB[:])
        wait_cc.__exit__(None, None, None)

        # ---------------- expansion (fp8 DoubleRow), CC-tolerant split ----
        # pass L: levels 4-7 (fully local) stream first; per group one DR
        # matmul -> PSUM -> fp8 partial in SBUF.  pass C: levels 0-3 ut
        # tiles stream second and stay resident, so when the collective
        # lands (barrier-gated, ~85-105us) only 64 short matmuls + adds
        # remain.  Final add y = yL + psC runs on DVE (direct) or
        # ACT-copy + gpsimd-add, alternating, to spread engine load.
        utap = ctx.enter_context(tc.tile_pool(name="utap", bufs=2))
        utbp = ctx.enter_context(tc.tile_pool(name="utbp", bufs=8))
        ylp = ctx.enter_context(tc.tile_pool(name="ylp", bufs=1))
        scrp = ctx.enter_context(tc.tile_pool(name="scrp", bufs=4))
        yL = [None] * NG
        with tc.tile_pool(name="epL", bufs=4, space="PSUM") as epL:
            for i in range(8):
                uta = utap.tile([128, 2, 4096], F8, tag="uta", name=f"uta{i}")
                for ff in range(2):
                    nc.sync.dma_start(
                        uta[:, ff, :], ut_d[2 + ff, :, 4096 * i : 4096 * (i + 1)]
                    )
                for gg in range(8):
                    g = 8 * i + gg
                    sl = slice(512 * gg, 512 * (gg + 1))
                    psL = epL.tile([B, 512], F32, tag="expL", name=f"psL{g}")
                    nc.tensor.matmul(
                        psL[:], statA[g // 2][:], uta[:, :, sl],
                        start=True, stop=True, perf_mode=DR,
                    )
                    yl = ylp.tile([B, 512], F8, tag=f"yL_{g}")
                    nc.vector.tensor_copy(yl[:], psL[:])
                    yL[g] = yl
        # pass C: levels 0-3; ut tiles stay resident in SBUF
        utb = []
        for i in range(8):
            t_ = utbp.tile([128, 2, 4096], F8, tag="utb", name=f"utb{i}")
            for ff in range(2):
                nc.sync.dma_start(
                    t_[:, ff, :], ut_d[ff, :, 4096 * i : 4096 * (i + 1)]
                )
            utb.append(t_)
        with tc.tile_pool(name="epC", bufs=4, space="PSUM") as epC, \
             tc.tile_wait_until(0.085):
            for i in range(8):
                y_t = yp.tile([B, 4096], F8, tag="y", name=f"y{i}")
                for gg in range(8):
                    g = 8 * i + gg
                    sl = slice(512 * gg, 512 * (gg + 1))
                    psC = epC.tile([B, 512], F32, tag="expC", name=f"psC{g}")
                    nc.tensor.matmul(
                        psC[:], statB[g // 32][:], utb[i][:, :, sl],
                        start=True, stop=True, perf_mode=DR,
                    )
                    if g % 2 == 0:
                        nc.vector.tensor_tensor(
                            y_t[:, sl], yL[g][:], psC[:], op=ADD
                        )
                    else:
                        sc = scrp.tile([B, 512], BF, tag="scr", name=f"scr{g}")
                        nc.scalar.copy(sc[:], psC[:])
                        nc.gpsimd.tensor_tensor(
                            y_t[:, sl], yL[g][:], sc[:], op=ADD
                        )
                nc.scalar.dma_start(
                    corr_d[:, 4096 * i : 4096 * (i + 1)], y_t[:]
                )

    nc.compile()
    return nc


def _pack_inputs(x, diag, u):
    """Build per-core input maps. x (B,N,1) f32, u (DEPTH,N,R) f32."""
    in_maps = []
    x2 = np.asarray(x).reshape(B, N)
    u3 = np.asarray(u)
    for c in range(NCORES):
        base = c * M
        xsl = x2[:, base : base + M]                      # (B, M)
        usl = u3[:, base : base + M, :] * USCALE          # (8, M, 64)
        xt = np.ascontiguousarray(
            xsl.T.reshape(CH, 128, B).transpose(1, 0, 2)
        ).astype(FP8)                                     # [128, CH, B]
        ua = np.ascontiguousarray(
            usl[0:3].transpose(1, 0, 2).reshape(M, 192)
            .reshape(CH, 128, 192).transpose(1, 0, 2)
        ).astype(FP8)                                     # [128, CH, 192]
        ub = np.ascontiguousarray(
            usl[3:8].transpose(1, 0, 2).reshape(M, 320)
            .reshape(CH, 128, 320).transpose(1, 0, 2)
        ).astype(FP8)                                     # [128, CH, 320]
        utp = np.ascontiguousarray(
            usl.transpose(0, 2, 1).reshape(4, 128, M)
        ).astype(FP8)                                     # [4, 128, M]
        # masks: mask[d, l] = 1 iff this core c is in the level-l sibling
        # block of destination core d.
        mA = np.zeros((128, 8, B), dtype=BF16)
        mB = np.zeros((64, 8, B), dtype=BF16)
        for d in range(8):
            if (c // 4) == ((d // 4) ^ 1):
                mA[0:64, d, :] = 1.0   # level 0
            if (c // 2) == ((d // 2) ^ 1):
                mA[64:128, d, :] = 1.0  # level 1
            if c == d ^ 1:
                mB[:, d, :] = 1.0       # level 2
        in_maps.append(
            {"xt": xt, "ua": ua, "ub": ub, "ut": utp, "maskA": mA, "maskB": mB}
        )
    return in_maps


last_results = None


def kernel(x, diag, u):
    global last_results
    from concourse.bass_utils import run_bass_kernel_spmd

    if "nc" not in _cached:
        _cached["nc"] = _build_bass()
    nc = _cached["nc"]

    in_maps = _pack_inputs(x, diag, u)
    res = run_bass_kernel_spmd(nc, in_maps, core_ids=list(range(NCORES)))
    last_results = res

    x2 = np.asarray(x, dtype=np.float32).reshape(B, N)
    d2 = np.asarray(diag, dtype=np.float32).reshape(1, N)
    y = d2 * x2
    inv = 1.0 / (USCALE * USCALE)
    for c in range(NCORES):
        corr = np.asarray(res.results[c]["corr"]).astype(np.float32)
        y[:, c * M : (c + 1) * M] += corr * inv
    return y.reshape(B, N, 1).astype(np.float32)


# revision 9
# speedup vs baseline: 1.4389x; 1.2037x over previous
"""HODLR matvec kernel for 8 TRN2 NeuronCores (Bass/Tile) — v2.

Sharding: node axis split into 8 contiguous slices of 32768 nodes.

v2 redesign (from v1 trace analysis: 186us, ~30us dead zone waiting on a
collective triggered at 58us + LDWEIGHTS-bound projection):

  projection  x is the STATIONARY operand (64-col weight loads, cheap);
              u streams as the moving operand, all levels at once per
              128-node chunk.  Two passes: levels 0-2 first (6.3MB of u)
              so the AllGather triggers at ~26us, then levels 3-7.
              Even/odd chunks run concurrently on col-groups (0,0)/(0,64)
              of the PE array; the halves are merged for free by the
              stacked-identity transpose matmuls.
  tree        bf16 lane-aligned adds on gpsimd (S2..S16 granularities).
  transpose   t^T[b,r] -> t[r,b] stationaries via normal matmuls with a
              stacked identity rhs ([I64;I64]), which also merges the
              even/odd col-group halves.
  expansion   fp8 DoubleRow matmuls, K=256 per instruction; per 512-node
              group one matmul over levels 0-3 (CC-gated) and one over
              levels 4-7 (local) accumulate into one PSUM tile.  ut
              streams interleaved (lvl 0-3 tile i, lvl 4-7 tile i, ...)
              so a late collective only stalls the PE, not the DMA pipe.

DMA priority order on the sync HWDGE queue: x -> u(l0-2) -> u(l3-7) ->
ut interleaved.  corr writes + CC recv ride the scalar queue; the masked
sibling combine runs on gpsimd (strict-FIFO-safe: trigger, tree adds,
combine are its only queue entries).
u/x are fed as fp8e4m3 (u scaled by USCALE; host divides the returned
corrections by USCALE^2).  Host computes diag*x in fp32 and adds.
"""

import sys

sys.path.insert(0, "/opt/trn_rl_repo")

import numpy as np
import ml_dtypes

BF16 = ml_dtypes.bfloat16
FP8 = ml_dtypes.float8_e4m3

B = 64
N = 262144
NCORES = 8
M = N // NCORES          # 32768 nodes per core
R = 64
DEPTH = 8
CH = M // 128            # 256 chunks of 128 nodes
NB7 = M // 1024          # 32 L7 blocks (1024 nodes each)
NG = M // 512            # 64 expansion groups of 512 nodes
USCALE = 64.0

_cached = {}


def _build_bass():
    import concourse.bacc as bacc
    import concourse.tile as tile
    import concourse.mybir as mybir
    from concourse.masks import make_identity
    from contextlib import ExitStack

    BF = mybir.dt.bfloat16
    F8 = mybir.dt.float8e4
    F32 = mybir.dt.float32
    ADD = mybir.AluOpType.add
    MULT = mybir.AluOpType.mult
    DR = mybir.MatmulPerfMode.DoubleRow

    nc = bacc.Bacc(
        "TRN2",
        target_bir_lowering=False,
        debug=False,
        enable_asserts=False,
        num_devices=NCORES,
    )

    xt_d = nc.dram_tensor("xt", [128, CH, B], F8, kind="ExternalInput").ap()
    ua_d = nc.dram_tensor("ua", [128, CH, 192], F8, kind="ExternalInput").ap()
    ub_d = nc.dram_tensor("ub", [128, CH, 320], F8, kind="ExternalInput").ap()
    ut_d = nc.dram_tensor("ut", [4, 128, M], F8, kind="ExternalInput").ap()
    mA_d = nc.dram_tensor("maskA", [128, 8, B], BF, kind="ExternalInput").ap()
    mB_d = nc.dram_tensor("maskB", [64, 8, B], BF, kind="ExternalInput").ap()
    corr_d = nc.dram_tensor("corr", [B, M], F8, kind="ExternalOutput").ap()

    with tile.TileContext(nc) as tc, ExitStack() as ctx:
        const = ctx.enter_context(tc.tile_pool(name="const", bufs=1))
        treep = ctx.enter_context(tc.tile_pool(name="treep", bufs=1))
        statp = ctx.enter_context(tc.tile_pool(name="statp", bufs=1))
        yp = ctx.enter_context(tc.tile_pool(name="yp", bufs=2))
        pp2 = ctx.enter_context(tc.tile_pool(name="pp2", bufs=2, space="PSUM"))
        tpp = ctx.enter_context(tc.tile_pool(name="tpp", bufs=2, space="PSUM"))
        dram = ctx.enter_context(tc.tile_pool(name="dram", bufs=1, space="DRAM"))

        # ---------------- constants + x ----------------
        xt = const.tile([128, CH, B], F8, tag="xt")
        for xq in range(4):
            nc.sync.dma_start(
                xt[:, 64 * xq : 64 * (xq + 1), :],
                xt_d[:, 64 * xq : 64 * (xq + 1), :],
            )
        # stacked identity [I64; I64]: transpose-merge rhs
        idS = const.tile([128, 64], BF, tag="idS")
        make_identity(nc, idS[0:64, :])
        make_identity(nc, idS[64:128, :])
        mA = const.tile([128, 8, B], BF, tag="mA")
        nc.scalar.dma_start(mA[:], mA_d[:])
        mB = const.tile([64, 8, B], BF, tag="mB")
        nc.scalar.dma_start(mB[:], mB_d[:])
        b_in = dram.tile([192, B], BF, tag="b_in")
        b_out = dram.tile([8, 192, B], BF, tag="b_out", addr_space="Shared")

        # round-robin for PSUM->SBUF drains (gpsimd cannot read PSUM)
        _cc = [0]

        def drain(out, in_):
            e = "vs"[_cc[0] % 2]
            _cc[0] += 1
            if e == "v":
                nc.vector.tensor_copy(out, in_)
            else:
                nc.scalar.copy(out, in_)

        # ---------------- pass 1: levels 0-2 projection ----------------
        # t012^T[b, l*64+r] accumulated over ALL chunks (levels 0-2 need
        # the full-slice contraction; sibling data comes via AllGather).
        with tc.tile_pool(name="pp1", bufs=1, space="PSUM") as pp1, \
             tc.tile_pool(name="uap", bufs=2) as uap:
            p1 = pp1.tile([128, 192], F32, tag="p1")
            UAT = 4
            for t_ in range(UAT):
                ua_t = uap.tile([128, 64, 192], F8, tag="ua", name=f"ua{t_}")
                if t_ == 0:
                    for sq in range(4):
                        nc.sync.dma_start(
                            ua_t[:, 16 * sq : 16 * (sq + 1), :],
                            ua_d[:, 16 * sq : 16 * (sq + 1), :],
                        )
                else:
                    nc.sync.dma_start(
                        ua_t[:], ua_d[:, 64 * t_ : 64 * (t_ + 1), :]
                    )
                for m in range(64):
                    ck = 64 * t_ + m
                    half = ck % 2
                    nc.tensor.matmul(
                        p1[64 * half : 64 * half + 64, :],
                        xt[:, ck, :],
                        ua_t[:, m, :],
                        start=(ck < 2),
                        stop=(ck >= CH - 2),
                    )
            s012 = treep.tile([128, 192], BF, tag="s012")
            with tc.high_priority():
                nc.vector.tensor_copy(s012[:], p1[:])

        # transpose-merge t012 -> b_in layout [l*64+r, b]; the whole
        # chain down to the CC trigger is high-priority so the Tile
        # scheduler does not defer any link behind pass-2 work.
        with tc.high_priority():
            pAB_t = tpp.tile([128, 2, 64], F32, tag="tp", name="tp_AB")
            pAB = pAB_t[:, 0, :]
            nc.tensor.matmul(pAB[0:64, :], s012[:, 0:64], idS[:], start=True, stop=True)
            nc.tensor.matmul(pAB[64:128, :], s012[:, 64:128], idS[:], start=True, stop=True)
            pC_t = tpp.tile([128, 2, 64], F32, tag="tp", name="tp_C")
            pC = pC_t[0:64, 0, :]
            nc.tensor.matmul(pC[:], s012[:, 128:192], idS[:], start=True, stop=True)
            bsA = treep.tile([128, B], BF, tag="bsA")
            nc.vector.tensor_copy(bsA[:], pAB[:])
            bsB = treep.tile([64, B], BF, tag="bsB")
            nc.vector.tensor_copy(bsB[:], pC[:])
            nc.scalar.dma_start(b_in[0:128, :], bsA[:])
            nc.scalar.dma_start(b_in[128:192, :], bsB[:])
            nc.gpsimd.collective_compute(
                "AllGather",
                mybir.AluOpType.bypass,
                replica_groups=[list(range(NCORES))],
                ins=[b_in.opt()],
                outs=[b_out.opt()],
            )

        # ---------------- pass 2: levels 3-7 projection + tree ----------
        # pass-2 column layout: l3@0:64 l4@64:128 l5@128:192 l6@192:256
        # l7@256:320.  s1[j] rows 0:64 = even-chunk partial of block j,
        # rows 64:128 = odd-chunk partial (merged later by transposes).
        S1, S2, S4, S8, S16 = [], [], [], [], []
        statA = [None] * NB7
        statB = [None] * 2

        def tree_add(lst, src, j, width, tag):
            g2 = treep.tile([128, width], BF, tag=f"{tag}_{j}")
            nc.gpsimd.tensor_tensor(
                g2[:], src[2 * j][:, 0:width], src[2 * j + 1][:, 0:width], op=ADD
            )
            lst.append(g2)

        def emit_statA(m7):
            # build [128, 2, B] fp8 stationary for levels 4-7 of block m7
            ps = tpp.tile([128, 2, 64], F32, tag="tp", name=f"tA4_{m7}")
            nc.tensor.matmul(
                ps[0:64, 0, :], S8[(m7 // 8) ^ 1][:, 64:128], idS[:],
                start=True, stop=True,
            )
            nc.tensor.matmul(
                ps[64:128, 0, :], S4[(m7 // 4) ^ 1][:, 128:192], idS[:],
                start=True, stop=True,
            )
            nc.tensor.matmul(
                ps[0:64, 1, :], S2[(m7 // 2) ^ 1][:, 192:256], idS[:],
                start=True, stop=True,
            )
            nc.tensor.matmul(
                ps[64:128, 1, :], S1[m7 ^ 1][:, 256:320], idS[:],
                start=True, stop=True,
            )
            s = statp.tile([128, 2, B], F8, tag=f"sA_{m7}")
            drain(s[:], ps[:])
            statA[m7] = s

        def emit_statB_l3(m3):
            # transpose l3 sibling into a half-stationary; tallA/tallB
            # columns are filled in after the collective combine.
            ps_t = tpp.tile([128, 2, 64], F32, tag="tp", name=f"tB3_{m3}")
            ps = ps_t[0:64, 0, :]
            nc.tensor.matmul(
                ps[:], S16[m3 ^ 1][:, 0:64], idS[:], start=True, stop=True
            )
            s = statp.tile([128, 2, B], F8, tag=f"sB_{m3}")
            nc.vector.tensor_copy(s[64:128, 1, :], ps[:])
            statB[m3] = s

        UBT = 8
        ubp = ctx.enter_context(tc.tile_pool(name="ubp", bufs=2))
        for t_ in range(UBT):
            ub_t = ubp.tile([128, 32, 320], F8, tag="ub", name=f"ub{t_}")
            nc.sync.dma_start(ub_t[:], ub_d[:, 32 * t_ : 32 * (t_ + 1), :])
            for blk in range(4):
                j = 4 * t_ + blk
                p2 = pp2.tile([128, 320], F32, tag="p2", name=f"p2_{j}")
                for ck8 in range(8):
                    ck = 8 * j + ck8
                    half = ck % 2
                    nc.tensor.matmul(
                        p2[64 * half : 64 * half + 64, :],
                        xt[:, ck, :],
                        ub_t[:, 8 * blk + ck8, :],
                        start=(ck8 < 2),
                        stop=(ck8 >= 6),
                    )
                s1 = treep.tile([128, 320], BF, tag=f"S1_{j}")
                drain(s1[:], p2[:])
                S1.append(s1)
                if j % 2 == 1:
                    tree_add(S2, S1, j // 2, 256, "S2")
                if j % 4 == 3:
                    tree_add(S4, S2, j // 4, 192, "S4")
                if j % 8 == 7:
                    tree_add(S8, S4, j // 8, 128, "S8")
                if j % 16 == 15:
                    tree_add(S16, S8, j // 16, 64, "S16")
                if j == 15:
                    # S8[0..1], S4[0..3], S2[0..7], S1[0..15] ready
                    for m7 in range(16):
                        emit_statA(m7)
                    emit_statB_l3(1)
                if j == 31:
                    for m7 in range(16, 32):
                        emit_statA(m7)
                    emit_statB_l3(0)


        # ---------------- expansion (fp8 DoubleRow), CC-tolerant split ----
        # pass L: levels 4-7 (fully local) stream first; per group one DR
        # matmul -> PSUM -> fp8 partial in SBUF.  pass C: levels 0-3 ut
        # tiles stream second and stay resident, so when the collective
        # lands (barrier-gated, ~85-105us) only 64 short matmuls + adds
        # remain.  Final add y = yL + psC runs on DVE (direct) or
        # ACT-copy + gpsimd-add, alternating, to spread engine load.
        utap = ctx.enter_context(tc.tile_pool(name="utap", bufs=2))
        utbp = ctx.enter_context(tc.tile_pool(name="utbp", bufs=8))
        ylp = ctx.enter_context(tc.tile_pool(name="ylp", bufs=1))
        scrp = ctx.enter_context(tc.tile_pool(name="scrp", bufs=4))
        yL = [None] * NG
        with tc.tile_pool(name="epL", bufs=4, space="PSUM") as epL:
            for i in range(8):
                uta = utap.tile([128, 2, 4096], F8, tag="uta", name=f"uta{i}")
                for ff in range(2):
                    nc.sync.dma_start(
                        uta[:, ff, :], ut_d[2 + ff, :, 4096 * i : 4096 * (i + 1)]
                    )
                for gg in range(8):
                    g = 8 * i + gg
                    sl = slice(512 * gg, 512 * (gg + 1))
                    psL = epL.tile([B, 512], F32, tag="expL", name=f"psL{g}")
                    nc.tensor.matmul(
                        psL[:], statA[g // 2][:], uta[:, :, sl],
                        start=True, stop=True, perf_mode=DR,
                    )
                    yl = ylp.tile([B, 512], F8, tag=f"yL_{g}")
                    nc.vector.tensor_copy(yl[:], psL[:])
                    yL[g] = yl
        # pass C: levels 0-3; ut tiles stay resident in SBUF
        utb = []
        for i in range(8):
            t_ = utbp.tile([128, 2, 4096], F8, tag="utb", name=f"utb{i}")
            for ff in range(2):
                nc.sync.dma_start(
                    t_[:, ff, :], ut_d[ff, :, 4096 * i : 4096 * (i + 1)]
                )
            utb.append(t_)
        # receive path + masked combine: emitted AFTER every ut DMA so
        # the CC-gated recv completions land last in the DMA lane
        # counters (otherwise the ut stream's WAR waits transitively
        # wait for the collective).
        with tc.tile_wait_until(0.085):
            recvA = statp.tile([128, 8, B], BF, tag="recvA")
            recvB = statp.tile([64, 8, B], BF, tag="recvB")
            for k in range(8):
                nc.scalar.dma_start(recvA[:, k, :], b_out[k, 0:128, :])
                nc.scalar.dma_start(recvB[:, k, :], b_out[k, 128:192, :])
        # masked sibling combine for levels 0-2 (gpsimd; after tree adds)
        wait_cc = tc.tile_wait_until(0.085)
        wait_cc.__enter__()
        mskA = statp.tile([128, 8, B], BF, tag="mskA")
        mskB = statp.tile([64, 8, B], BF, tag="mskB")
        nc.gpsimd.tensor_tensor(mskA[:], recvA[:], mA[:], op=MULT)
        nc.gpsimd.tensor_tensor(mskB[:], recvB[:], mB[:], op=MULT)
        nc.gpsimd.tensor_tensor(
            mskA[:, 0:4, :], mskA[:, 0:4, :], mskA[:, 4:8, :], op=ADD
        )
        nc.gpsimd.tensor_tensor(
            mskB[:, 0:4, :], mskB[:, 0:4, :], mskB[:, 4:8, :], op=ADD
        )
        nc.gpsimd.tensor_tensor(
            mskA[:, 0:2, :], mskA[:, 0:2, :], mskA[:, 2:4, :], op=ADD
        )
        nc.gpsimd.tensor_tensor(
            mskB[:, 0:2, :], mskB[:, 0:2, :], mskB[:, 2:4, :], op=ADD
        )
        tallA = statp.tile([128, B], BF, tag="tallA")
        tallB = statp.tile([64, B], BF, tag="tallB")
        nc.gpsimd.tensor_tensor(tallA[:], mskA[:, 0, :], mskA[:, 1, :], op=ADD)
        nc.gpsimd.tensor_tensor(tallB[:], mskB[:, 0, :], mskB[:, 1, :], op=ADD)
        for m3 in range(2):
            nc.scalar.copy(statB[m3][:, 0, :], tallA[:])
            nc.scalar.copy(statB[m3][0:64, 1, :], tallB[:])
        wait_cc.__exit__(None, None, None)

        with tc.tile_pool(name="epC", bufs=4, space="PSUM") as epC, \
             tc.tile_wait_until(0.085):
            for i in range(8):
                y_t = yp.tile([B, 4096], F8, tag="y", name=f"y{i}")
                for gg in range(8):
                    g = 8 * i + gg
                    sl = slice(512 * gg, 512 * (gg + 1))
                    psC = epC.tile([B, 512], F32, tag="expC", name=f"psC{g}")
                    nc.tensor.matmul(
                        psC[:], statB[g // 32][:], utb[i][:, :, sl],
                        start=True, stop=True, perf_mode=DR,
                    )
                    if g % 2 == 0:
                        nc.vector.tensor_tensor(
                            y_t[:, sl], yL[g][:], psC[:], op=ADD
                        )
                    else:
                        sc = scrp.tile([B, 512], BF, tag="scr", name=f"scr{g}")
                        nc.scalar.copy(sc[:], psC[:])
                        nc.gpsimd.tensor_tensor(
                            y_t[:, sl], yL[g][:], sc[:], op=ADD
                        )
                nc.scalar.dma_start(
                    corr_d[:, 4096 * i : 4096 * (i + 1)], y_t[:]
                )

    nc.compile()
    return nc


def _pack_inputs(x, diag, u):
    """Build per-core input maps. x (B,N,1) f32, u (DEPTH,N,R) f32."""
    in_maps = []
    x2 = np.asarray(x).reshape(B, N)
    u3 = np.asarray(u)
    for c in range(NCORES):
        base = c * M
        xsl = x2[:, base : base + M]                      # (B, M)
        usl = u3[:, base : base + M, :] * USCALE          # (8, M, 64)
        xt = np.ascontiguousarray(
            xsl.T.reshape(CH, 128, B).transpose(1, 0, 2)
        ).astype(FP8)                                     # [128, CH, B]
        ua = np.ascontiguousarray(
            usl[0:3].transpose(1, 0, 2).reshape(M, 192)
            .reshape(CH, 128, 192).transpose(1, 0, 2)
        ).astype(FP8)                                     # [128, CH, 192]
        ub = np.ascontiguousarray(
            usl[3:8].transpose(1, 0, 2).reshape(M, 320)
            .reshape(CH, 128, 320).transpose(1, 0, 2)
        ).astype(FP8)                                     # [128, CH, 320]
        utp = np.ascontiguousarray(
            usl.transpose(0, 2, 1).reshape(4, 128, M)
        ).astype(FP8)                                     # [4, 128, M]
        # masks: mask[d, l] = 1 iff this core c is in the level-l sibling
        # block of destination core d.
        mA = np.zeros((128, 8, B), dtype=BF16)
        mB = np.zeros((64, 8, B), dtype=BF16)
        for d in range(8):
            if (c // 4) == ((d // 4) ^ 1):
                mA[0:64, d, :] = 1.0   # level 0
            if (c // 2) == ((d // 2) ^ 1):
                mA[64:128, d, :] = 1.0  # level 1
            if c == d ^ 1:
                mB[:, d, :] = 1.0       # level 2
        in_maps.append(
            {"xt": xt, "ua": ua, "ub": ub, "ut": utp, "maskA": mA, "maskB": mB}
        )
    return in_maps


last_results = None


def kernel(x, diag, u):
    global last_results
    from concourse.bass_utils import run_bass_kernel_spmd

    if "nc" not in _cached:
        _cached["nc"] = _build_bass()
    nc = _cached["nc"]

    in_maps = _pack_inputs(x, diag, u)
    res = run_bass_kernel_spmd(nc, in_maps, core_ids=list(range(NCORES)))
    last_results = res

    x2 = np.asarray(x, dtype=np.float32).reshape(B, N)
    d2 = np.asarray(diag, dtype=np.float32).reshape(1, N)
    y = d2 * x2
    inv = 1.0 / (USCALE * USCALE)
    for c in range(NCORES):
        corr = np.asarray(res.results[c]["corr"]).astype(np.float32)
        y[:, c * M : (c + 1) * M] += corr * inv
    return y.reshape(B, N, 1).astype(np.float32)
